# revision 14
# baseline (speedup 1.0000x reference)
"""CARAFE-naive upsampling (N=4, C=256, H=W=64, k=5, g=4, s=2) on 8 TRN2
NeuronCores.

Strategy
--------
Sharding: core c <- (batch n = c//2, group-pair j = c%2). Each core owns 128
feature channels (2 of the 4 mask groups) of one batch image.

Compute: the per-pixel mask application is reformulated as TensorEngine
matmuls. For one source row r and a w-tile of 32 source columns:

    out[(g,c), (a,w,b)] += sum_{w'} statT[(g,w'), (g,c)] * B[(g,w'), (a,w,b)]

where statT is the (block-diagonal over the 2 groups) transposed feature row
and B is a *banded* matrix holding mask values on shifted diagonals
(row w+dj pairs source column w0+w+dj-2 with output column w). The 5 row
offsets di accumulate into PSUM (start/stop accumulation groups).

B cannot be built on-device (its diagonal layout is not an affine access
pattern), so the host pre-shears masks into B in numpy and ships it to HBM
in matmul-ready bf16 layout. bf16 inflation is 7.2x over raw masks but the
TensorEngine then does all 52M MACs/core in ~628 matmuls.
"""

import sys

import numpy as np

for _p in ("/opt/trn_rl_repo", "/opt/pypackages"):
    if _p not in sys.path:
        sys.path.append(_p)

import ml_dtypes  # noqa: E402
from contextlib import ExitStack  # noqa: E402

import concourse.bass as bass  # noqa: E402
import concourse.tile as tile  # noqa: E402
from concourse import bacc, mybir  # noqa: E402
from concourse.bass_utils import run_bass_kernel_spmd  # noqa: E402

# Problem constants (hardcoded per harness contract)
KS = 5            # kernel size
G = 4             # mask groups
S = 2             # upscale
N, C, H, W = 4, 256, 64, 64
Wt = 32           # w-tile
NT = W // Wt      # 2 tiles
KB = Wt + 4       # band rows per group
KK = 2 * KB       # contraction dim = 72
BF16 = ml_dtypes.bfloat16

_NC_CACHE = {}


def _build_bass():
    # Bacc (not raw Bass): its finalize() runs generate_event_semaphores,
    # which splits multi-sem waits to satisfy the 1-wait-per-instruction
    # TRN2 ISA constraint.
    nc = bacc.Bacc()
    stat_d = nc.declare_dram_parameter(
        "stat", [H, NT, KK, 128], mybir.dt.bfloat16, isOutput=False)
    bmat_d = nc.declare_dram_parameter(
        "bmat", [H, KS, NT, KK, 128], mybir.dt.bfloat16, isOutput=False)
    out_d = nc.declare_dram_parameter(
        "out", [128, S * H, S * W], mybir.dt.float32, isOutput=True)

    # (a, t, w, b) order: strides a=128, t=64, (w b)=1 — merges to 3 DMA dims
    out_view = out_d.rearrange(
        "c (h a) (t w b) -> c h a t w b", a=S, t=NT, b=S)

    with tile.TileContext(nc) as tc, ExitStack() as ctx:
        statp = ctx.enter_context(tc.tile_pool(name="statp", bufs=1))
        bp = ctx.enter_context(tc.tile_pool(name="bp", bufs=8))
        pp = ctx.enter_context(tc.tile_pool(name="pp", bufs=8, space="PSUM"))
        op = ctx.enter_context(tc.tile_pool(name="op", bufs=4))

        # All 128 stationaries resident in SBUF: [72, 64, 2, 128] bf16 = 2.36MB
        stat_all = statp.tile([KK, H, NT, 128], mybir.dt.bfloat16)
        nc.sync.dma_start(out=stat_all, in_=stat_d.rearrange("r t k m -> k r t m"))

        btiles = {}
        psums = {}

        def rfirst(h):
            return max(0, h - 2)

        def rlast(h):
            return min(H - 1, h + 2)

        for r in range(H):
            # prefetch B batches entering the window
            for h in range(max(0, r - 2), min(H - 1, r + 2) + 1):
                if h not in btiles:
                    bt = bp.tile([KK, KS, NT, 128], mybir.dt.bfloat16,
                                 name=f"bt{h}", tag="bt")
                    # SWDGE (gpsimd): DMA trigger waits run in ucode, so the
                    # WAR wait on slot reuse + queue credit both fit.
                    nc.gpsimd.dma_start(
                        out=bt, in_=bmat_d[h].rearrange("di t k m -> k di t m"))
                    btiles[h] = bt

            for di in range(KS):
                h = r + 2 - di
                if not (0 <= h < H):
                    continue
                if h not in psums:
                    psums[h] = pp.tile([128, NT, 128], mybir.dt.float32,
                                       name=f"ps{h}", tag="ps")
                first = r == rfirst(h)
                last = r == rlast(h)
                for t in range(NT):
                    nc.tensor.matmul(
                        out=psums[h][:, t, :],
                        lhsT=stat_all[:, r, t, :],
                        rhs=btiles[h][:, di, t, :],
                        start=(first and t == 0),
                        stop=(last and t == NT - 1),
                        skip_group_check=True,
                    )

            # drain finished output rows
            done = [r - 2] if r - 2 >= 0 else []
            if r == H - 1:
                done += [H - 2, H - 1]
            for h in done:
                ot = op.tile([128, S, NT, Wt, S], mybir.dt.float32)
                ot_tawb = ot.rearrange("c a t w b -> c t a w b")
                ps_tawb = psums[h].rearrange("c t (a w b) -> c t a w b", a=S, b=S)
                # copy + DMA both on ACT: producer and WAR deps become
                # program order, leaving each instruction <= 1 sem wait
                nc.scalar.copy(out=ot_tawb, in_=ps_tawb)
                nc.scalar.dma_start(out=out_view[:, h], in_=ot)
                del psums[h], btiles[h]

    nc.finalize()
    return nc


def _host_shards(features, masks):
    """Build per-core stat/bmat arrays (bf16)."""
    in_maps = []
    iw = np.arange(Wt)
    for c in range(8):
        n, j = c // 2, c % 2
        f = features[n, 128 * j: 128 * (j + 1)]        # [128, 64, 64] f32
        m = masks[n, 50 * j: 50 * j + 50]              # [50, 128, 128] f32

        # stationaries: stat[r, t, g*KB + w', g*64 + cc] = fpad[g*64+cc, r, 32t + w']
        stat = np.zeros((H, NT, KK, 128), np.float32)
        fp = np.pad(f, ((0, 0), (0, 0), (2, 2)))
        for g in range(2):
            for t in range(NT):
                sl = fp[g * 64:(g + 1) * 64, :, Wt * t: Wt * t + KB]
                stat[:, t, g * KB:(g + 1) * KB, g * 64:(g + 1) * 64] = \
                    sl.transpose(1, 2, 0)

        # banded masks: B[h, di, t, g*KB + w + dj, (a,w,b)]
        M8 = m.reshape(2, KS, KS, H, S, NT, Wt, S)     # g,di,dj,h,a,t,w,b
        B2 = np.zeros((H, KS, NT, S, S, KK, Wt), np.float32)
        for g in range(2):
            for dj in range(KS):
                src = M8[g, :, dj].transpose(1, 0, 3, 2, 5, 4)  # h,di,t,a,b,w
                B2[:, :, :, :, :, g * KB + iw + dj, iw] = src
        bmat = np.ascontiguousarray(
            B2.transpose(0, 1, 2, 5, 3, 6, 4)).reshape(H, KS, NT, KK, 128)

        in_maps.append({
            "stat": np.ascontiguousarray(stat).astype(BF16),
            "bmat": bmat.astype(BF16),
        })
    return in_maps


def kernel(features, masks, _trace=False):
    features = np.asarray(features, dtype=np.float32)
    masks = np.asarray(masks, dtype=np.float32)

    in_maps = _host_shards(features, masks)

    if "nc" not in _NC_CACHE:
        _NC_CACHE["nc"] = _build_bass()
    nc = _NC_CACHE["nc"]

    res = run_bass_kernel_spmd(nc, in_maps, list(range(8)), trace=_trace)
    kernel._last_result = res

    out = np.empty((N, C, S * H, S * W), np.float32)
    for c in range(8):
        n, j = c // 2, c % 2
        out[n, 128 * j: 128 * (j + 1)] = res.results[c]["out"]
    return out


# revision 18
# speedup vs baseline: 1.3729x; 1.3729x over previous
"""CARAFE-naive upsampling (N=4, C=256, H=W=64, k=5, g=4, s=2) on 8 TRN2
NeuronCores.

Strategy
--------
Sharding: core c <- (batch n = c//2, group-pair j = c%2). Each core owns 128
feature channels (2 of the 4 mask groups) of one batch image.

Compute: the per-pixel mask application is reformulated as TensorEngine
matmuls. For one source row r and a w-tile of 32 source columns:

    out[(g,c), (a,w,b)] += sum_{w'} statT[(g,w'), (g,c)] * B[(g,w'), (a,w,b)]

where statT is the (block-diagonal over the 2 groups) transposed feature row
and B is a *banded* matrix holding mask values on shifted diagonals
(row w+dj pairs source column w0+w+dj-2 with output column w). The 5 row
offsets di accumulate into PSUM (start/stop accumulation groups).

B cannot be built on-device (its diagonal layout is not an affine access
pattern), so the host pre-shears masks into B in numpy and ships it to HBM
in matmul-ready bf16 layout. bf16 inflation is 7.2x over raw masks but the
TensorEngine then does all 52M MACs/core in ~628 matmuls.
"""

import sys

import numpy as np

for _p in ("/opt/trn_rl_repo", "/opt/pypackages"):
    if _p not in sys.path:
        sys.path.append(_p)

import ml_dtypes  # noqa: E402
from contextlib import ExitStack  # noqa: E402

import concourse.bass as bass  # noqa: E402
import concourse.tile as tile  # noqa: E402
from concourse import bacc, mybir  # noqa: E402
from concourse.bass_utils import run_bass_kernel_spmd  # noqa: E402

# Problem constants (hardcoded per harness contract)
KS = 5            # kernel size
G = 4             # mask groups
S = 2             # upscale
N, C, H, W = 4, 256, 64, 64
Wt = 32           # w-tile
NT = W // Wt      # 2 tiles
KB = Wt + 4       # band rows per group
KK = 2 * KB       # contraction dim = 72
BF16 = ml_dtypes.bfloat16

_NC_CACHE = {}


def _build_bass():
    # Bacc (not raw Bass): its finalize() runs generate_event_semaphores,
    # which splits multi-sem waits to satisfy the 1-wait-per-instruction
    # TRN2 ISA constraint.
    nc = bacc.Bacc()
    # k-major layouts: every DMA walks contiguous bytes per SBUF partition
    stat_d = nc.declare_dram_parameter(
        "stat", [KK, H, NT, 128], mybir.dt.bfloat16, isOutput=False)
    bmat_d = nc.declare_dram_parameter(
        "bmat", [H, KK, KS, NT, 128], mybir.dt.bfloat16, isOutput=False)
    out_d = nc.declare_dram_parameter(
        "out", [128, S * H, S * W], mybir.dt.bfloat16, isOutput=True)

    HB = 8  # output rows per batched store
    out_rows = out_d.rearrange("c (hb y) x -> c hb (y x)", hb=H // HB)

    with tile.TileContext(nc) as tc, ExitStack() as ctx:
        statp = ctx.enter_context(tc.tile_pool(name="statp", bufs=1))
        bp = ctx.enter_context(tc.tile_pool(name="bp", bufs=8))
        pp = ctx.enter_context(tc.tile_pool(name="pp", bufs=8, space="PSUM"))
        op = ctx.enter_context(tc.tile_pool(name="op", bufs=3))

        # All 128 stationaries resident in SBUF: [72, 64, 2, 128] bf16 = 2.36MB
        stat_all = statp.tile([KK, H, NT, 128], mybir.dt.bfloat16)
        nc.sync.dma_start(out=stat_all, in_=stat_d[:])

        btiles = {}
        psums = {}
        otiles = {}

        def rfirst(h):
            return max(0, h - 2)

        def rlast(h):
            return min(H - 1, h + 2)

        for r in range(H):
            # prefetch B batches entering the window (contiguous per-partition)
            for h in range(max(0, r - 2), min(H - 1, r + 2) + 1):
                if h not in btiles:
                    bt = bp.tile([KK, KS, NT, 128], mybir.dt.bfloat16,
                                 name=f"bt{h}", tag="bt")
                    nc.sync.dma_start(out=bt, in_=bmat_d[h])
                    btiles[h] = bt

            # t-outer: 5 consecutive matmuls share one stationary
            for t in range(NT):
                for di in range(KS):
                    h = r + 2 - di
                    if not (0 <= h < H):
                        continue
                    if h not in psums:
                        psums[h] = pp.tile([128, NT, 128], mybir.dt.float32,
                                           name=f"ps{h}", tag="ps")
                    nc.tensor.matmul(
                        out=psums[h][:, t, :],
                        lhsT=stat_all[:, r, t, :],
                        rhs=btiles[h][:, di, t, :],
                        start=(r == rfirst(h) and t == 0),
                        stop=(r == rlast(h) and t == NT - 1),
                        skip_group_check=True,
                    )

            # drain finished output rows into an 8-row staging buffer
            done = [r - 2] if r - 2 >= 0 else []
            if r == H - 1:
                done += [H - 2, H - 1]
            for h in done:
                blk = h // HB
                if blk not in otiles:
                    otiles[blk] = op.tile([128, HB, S, NT, Wt, S],
                                          mybir.dt.bfloat16,
                                          name=f"ot{blk}", tag="ot")
                ot_tawb = otiles[blk][:, h % HB].rearrange(
                    "c a t w b -> c t a w b")
                ps_tawb = psums[h].rearrange(
                    "c t (a w b) -> c t a w b", a=S, b=S)
                if h % 2 == 0:
                    nc.vector.tensor_copy(out=ot_tawb, in_=ps_tawb)
                else:
                    nc.scalar.copy(out=ot_tawb, in_=ps_tawb)
                del psums[h], btiles[h]
                if h % HB == HB - 1:
                    # 4KB/partition fully-contiguous store
                    nc.scalar.dma_start(
                        out=out_rows[:, blk], in_=otiles[blk])
                    del otiles[blk]

    nc.finalize()
    return nc


def _host_shards(features, masks):
    """Build per-core stat/bmat arrays (bf16)."""
    in_maps = []
    iw = np.arange(Wt)
    for c in range(8):
        n, j = c // 2, c % 2
        f = features[n, 128 * j: 128 * (j + 1)]        # [128, 64, 64] f32
        m = masks[n, 50 * j: 50 * j + 50]              # [50, 128, 128] f32

        # stationaries: stat[g*KB + w', r, t, g*64 + cc] = fpad[g*64+cc, r, 32t+w']
        stat = np.zeros((KK, H, NT, 128), np.float32)
        fp = np.pad(f, ((0, 0), (0, 0), (2, 2)))
        for g in range(2):
            for t in range(NT):
                sl = fp[g * 64:(g + 1) * 64, :, Wt * t: Wt * t + KB]
                stat[g * KB:(g + 1) * KB, :, t, g * 64:(g + 1) * 64] = \
                    sl.transpose(2, 1, 0)

        # banded masks: B[h, g*KB + w + dj, di, t, (a,w,b)]
        M8 = m.reshape(2, KS, KS, H, S, NT, Wt, S)     # g,di,dj,h,a,t,w,b
        B2 = np.zeros((H, KS, NT, S, S, KK, Wt), np.float32)
        for g in range(2):
            for dj in range(KS):
                src = M8[g, :, dj].transpose(1, 0, 3, 2, 5, 4)  # h,di,t,a,b,w
                B2[:, :, :, :, :, g * KB + iw + dj, iw] = src
        bmat = np.ascontiguousarray(
            B2.transpose(0, 5, 1, 2, 3, 6, 4)).reshape(H, KK, KS, NT, 128)

        in_maps.append({
            "stat": np.ascontiguousarray(stat).astype(BF16),
            "bmat": bmat.astype(BF16),
        })
    return in_maps


def kernel(features, masks, _trace=False):
    features = np.asarray(features, dtype=np.float32)
    masks = np.asarray(masks, dtype=np.float32)

    in_maps = _host_shards(features, masks)

    if "nc" not in _NC_CACHE:
        _NC_CACHE["nc"] = _build_bass()
    nc = _NC_CACHE["nc"]

    res = run_bass_kernel_spmd(nc, in_maps, list(range(8)), trace=_trace)
    kernel._last_result = res

    out = np.empty((N, C, S * H, S * W), np.float32)
    for c in range(8):
        n, j = c // 2, c % 2
        out[n, 128 * j: 128 * (j + 1)] = \
            res.results[c]["out"].astype(np.float32)
    return out
